# revision 41
# baseline (speedup 1.0000x reference)
"""GCN encoder (3x GCNConv sharing one normalized adjacency) on 8 TRN2 NeuronCores.

v3 design:
  - Fold sym-norm into per-node scales: pre-scale rows by dis, post-scale
    aggregates by dis[dst].
  - Conv1 gathers directly from a replicated row-major (x*dis) table in HBM
    and aggregates raw input rows TRANSPOSED (psum[feat,dst] += chunk.T@OH);
    W1 is applied once per dst tile afterwards. No dense pre-GEMM, no first
    AllGather.
  - dma_gather descriptor generation runs on one Q7 core pair per SWDGE
    queue (~7.9ns/desc); gathers rotate over 4 queues so 4 pairs generate
    concurrently. 256B random HBM reads then become the wall (~0.35-0.5
    accesses/ns); deep buffering (TB=2 tile batches, 6 gather bufs) keeps
    the SDMA queues full, and per-block source-sorting improves locality.
  - One-hot scatter matrices precomputed on the host in fp8e4 and streamed
    from HBM (no DVE is_equal).
  - Self loops leave the gather streams; each dst tile adds its local rows
    via one identity matmul.
  - The republish AllGather is split in two chunks (tiles 0-24 / 25-48 of
    each shard) so chunk A overlaps the tail of conv1 and pass-2 gathers on
    table A overlap AllGather B. Pass 2 has its own group split (by chunk
    table), idx streams, and one-hots.
  - mu and logstd share one pass: Wc = [W_mu | W_logstd].
"""

import numpy as np
import ml_dtypes

N = 50000
E = 800000
IN = 128
HID = 128
OUT = 64
NCORES = 8
SH = 6272                 # nodes per core (padded)
NPAD = SH * NCORES        # 50176
NT = SH // 128            # 49 dst tiles per core
LO = 32768                # rows in pass-1 "lo" table (int16 limit)
TSPLIT = 31               # pass-2 chunk A = tiles [0,31), B = [31,49)
RA = TSPLIT * 128         # 3200 rows per shard in chunk A
RB = SH - RA              # 3072 rows per shard in chunk B
NROWA = NCORES * RA       # 25600 (< 32767: int16 ok)
NROWB = NCORES * RB       # 24576
TB = 1                    # dst tiles per gather batch
NQ = 4                    # SWDGE queues
AGA_AT = 36               # issue AllGather-A after consuming this tile (pass 1)
STAG = 10                 # pass-2: issue gB(t) after gA(t+STAG-1)

TRACE = False             # test.py sets this for profiling runs
LAST_RESULTS = None       # test.py reads exec_time_ns from here

_CACHE = {}


def _build_streams(es_tab, t, dl, g, ngrp):
    """Build per-core padded gather streams + fp8 one-hots for one pass.

    es_tab: per-message index into its group's table
    t: dst tile; dl: dst lane; g: group id (0..ngrp-1)
    All arrays are lists per core. Returns dict with C [NT,ngrp], offsets,
    per-core idx streams (per group) and OH fp8 arrays.
    """
    cnts = np.zeros((NCORES, NT, ngrp), np.int64)
    ordered = []
    for c in range(NCORES):
        order = np.lexsort((es_tab[c], g[c], t[c]))  # by tile, grp, src (locality)
        e, tt, dd, gg = es_tab[c][order], t[c][order], dl[c][order], g[c][order]
        key = tt * ngrp + gg
        bc = np.bincount(key, minlength=NT * ngrp)
        cnts[c] = bc.reshape(NT, ngrp)
        ordered.append((e, tt, dd, gg, key))

    C = (cnts.max(axis=0) + 127) // 128            # [NT, ngrp]
    K = C.sum(axis=0).astype(np.int64)             # chunks per group stream
    KT = int(C.sum())
    g_off = np.concatenate([np.zeros((1, ngrp), np.int64),
                            np.cumsum(C, axis=0)[:-1]], axis=0)  # [NT, ngrp]
    kk_off = np.concatenate([[0], np.cumsum(C.sum(axis=1))[:-1]])

    per_core = []
    for c in range(NCORES):
        e, tt, dd, gg, key = ordered[c]
        blk_start = np.concatenate([[0], np.cumsum(cnts[c].reshape(-1))[:-1]])
        rank = np.arange(len(e)) - blk_start[key]
        pos = g_off[tt, gg] * 128 + rank           # position in group stream
        streams = []
        for gi in range(ngrp):
            s = np.zeros(int(K[gi]) * 128, np.int16)
            m = gg == gi
            s[pos[m]] = e[m].astype(np.int16)
            streams.append(np.tile(s.reshape(-1, 16).T, (8, 1)))  # [128, K*8]
        # chunk index: tile-major, groups in order within tile
        cum_in_tile = np.concatenate(
            [np.zeros((NT, 1), np.int64), np.cumsum(C, axis=1)[:, :-1]], axis=1)
        kk = kk_off[tt] + cum_in_tile[tt, gg] + rank // 128
        oh = np.zeros((128, KT * 128), np.uint8)
        oh[rank % 128, kk * 128 + dd] = 0x38       # 1.0 in fp8 e4m3
        per_core.append((streams, oh.view(ml_dtypes.float8_e4m3)))

    return dict(C=C, K=K, KT=KT, g_off=g_off, kk_off=kk_off,
                per_core=per_core, ngrp=ngrp)


def _preprocess(edge_index):
    src = np.asarray(edge_index[0]).astype(np.int64)
    dst = np.asarray(edge_index[1]).astype(np.int64)
    loop = np.arange(N, dtype=np.int64)
    dst_all = np.concatenate([dst, loop])

    deg = np.bincount(dst_all, minlength=N).astype(np.float32)
    dis = (1.0 / np.sqrt(deg)).astype(np.float32)

    es_by_core, t_by_core, dl_by_core = [], [], []
    for c in range(NCORES):
        m = (dst // SH) == c
        es = src[m]
        ed = dst[m] - c * SH
        es_by_core.append(es)
        t_by_core.append(ed >> 7)
        dl_by_core.append(ed & 127)

    # pass 1: table = x2R [NPAD,128]; groups lo (idx<LO) / hi
    g1 = [(e >= LO).astype(np.int64) for e in es_by_core]
    e1 = [np.where(e >= LO, e - LO, e) for e in es_by_core]
    p1 = _build_streams(e1, t_by_core, dl_by_core, g1, 2)

    # pass 2: tables hcfA [NROWA,128] / hcfB [NROWB,128]
    # node (c2, r) -> table A pos c2*RA + r if r < RA else B pos c2*RB + (r-RA)
    g2, e2 = [], []
    for c in range(NCORES):
        e = es_by_core[c]
        c2, r = e // SH, e % SH
        inB = (r >= RA).astype(np.int64)
        g2.append(inB)
        e2.append(np.where(inB == 1, c2 * RB + (r - RA), c2 * RA + r))
    p2 = _build_streams(e2, t_by_core, dl_by_core, g2, 2)

    batches = []
    t0 = 0
    while t0 < NT:
        t1 = min(t0 + TB, NT)
        batches.append((t0, t1))
        t0 = t1
    return dis, dict(p1=p1, p2=p2, batches=batches)


def _build_nc(meta):
    import concourse.bass as bass
    import concourse.bacc as bacc
    import concourse.mybir as mybir
    import concourse.tile as tile
    from concourse import library_config

    batches = meta["batches"]
    p1, p2 = meta["p1"], meta["p2"]

    f16 = mybir.dt.float16
    f32 = mybir.dt.float32
    f8 = mybir.dt.float8e4
    i16 = mybir.dt.int16
    mult = mybir.AluOpType.mult
    add = mybir.AluOpType.add

    nc = bacc.Bacc("TRN2", target_bir_lowering=False, debug=False,
                   enable_asserts=True, num_devices=NCORES,
                   num_swdge_queues=NQ)

    x2Rd = nc.dram_tensor("x2Rd", [NPAD, 128], f16, kind="ExternalInput")
    xlocd = nc.dram_tensor("xlocd", [SH, 128], f16, kind="ExternalInput")
    W1d = nc.dram_tensor("W1d", [128, 128], f16, kind="ExternalInput")
    Wcd = nc.dram_tensor("Wcd", [128, 128], f16, kind="ExternalInput")
    b1rd = nc.dram_tensor("b1rd", [128, 128], f32, kind="ExternalInput")
    bcrd = nc.dram_tensor("bcrd", [128, 128], f32, kind="ExternalInput")
    disT32d = nc.dram_tensor("disT32d", [128, NT], f32, kind="ExternalInput")
    identd = nc.dram_tensor("identd", [128, 128], f8, kind="ExternalInput")
    idx1 = [nc.dram_tensor(f"idx1g{g}", [128, int(p1["K"][g]) * 8], i16,
                           kind="ExternalInput") for g in range(2)]
    idx2 = [nc.dram_tensor(f"idx2g{g}", [128, int(p2["K"][g]) * 8], i16,
                           kind="ExternalInput") for g in range(2)]
    oh1d = nc.dram_tensor("oh1d", [128, p1["KT"] * 128], f8, kind="ExternalInput")
    oh2d = nc.dram_tensor("oh2d", [128, p2["KT"] * 128], f8, kind="ExternalInput")
    out_ml = nc.dram_tensor("out_ml", [SH, 128], f32, kind="ExternalOutput")

    with tile.TileContext(nc) as tc:
        with (
            tc.tile_pool(name="consts", bufs=1) as cpool,
            tc.tile_pool(name="work", bufs=4) as wpool,
            tc.tile_pool(name="oh", bufs=8) as ohpool,
            tc.tile_pool(name="g0", bufs=12) as gpool0,
            tc.tile_pool(name="g1", bufs=12) as gpool1,
            tc.tile_pool(name="psA", bufs=6, space="PSUM") as psA,
            tc.tile_pool(name="psH", bufs=2, space="PSUM") as psH,
            tc.tile_pool(name="dram", bufs=1, space="DRAM") as dpool,
        ):
            nc.gpsimd.load_library(library_config.mlp)

            W1sb = cpool.tile([128, 128], f16, tag="W1sb")
            Wcsb = cpool.tile([128, 128], f16, tag="Wcsb")
            b1sb = cpool.tile([128, 128], f32, tag="b1sb")
            bcsb = cpool.tile([128, 128], f32, tag="bcsb")
            dis32sb = cpool.tile([128, NT], f32, tag="dis32sb")
            identsb = cpool.tile([128, 128], f8, tag="identsb")
            idx1sb = [cpool.tile([128, int(p1["K"][g]) * 8], i16,
                                 tag=f"idx1g{g}", name=f"idx1sb{g}")
                      for g in range(2)]
            idx2sb = [cpool.tile([128, int(p2["K"][g]) * 8], i16,
                                 tag=f"idx2g{g}", name=f"idx2sb{g}")
                      for g in range(2)]

            xres = cpool.tile([128, NT * 128], f16, tag="xres")
            hsres = cpool.tile([128, NT * 128], f16, tag="hsres")

            nc.sync.dma_start(W1sb[:], W1d.ap())
            nc.sync.dma_start(Wcsb[:], Wcd.ap())
            nc.sync.dma_start(b1sb[:], b1rd.ap())
            nc.sync.dma_start(bcsb[:], bcrd.ap())
            nc.sync.dma_start(dis32sb[:], disT32d.ap())
            nc.sync.dma_start(identsb[:], identd.ap())
            for g in range(2):
                nc.sync.dma_start(idx1sb[g][:], idx1[g].ap())
                nc.sync.dma_start(idx2sb[g][:], idx2[g].ap())
            for t in range(NT):
                nc.sync.dma_start(xres[:, t * 128:(t + 1) * 128],
                                  xlocd[t * 128:(t + 1) * 128, :])

            hcsA = dpool.tile([RA, 128], f16, tag="hcsA")
            hcsB = dpool.tile([RB, 128], f16, tag="hcsB")
            hcfA = dpool.tile([NROWA, 128], f16, tag="hcfA", addr_space="Shared")
            hcfB = dpool.tile([NROWB, 128], f16, tag="hcfB", addr_space="Shared")

            def conv_pass(pp, tables, idxsb, ohd_t, loc_res, is_conv1,
                          mid_cb=None, stag=0):
                C, g_off, kk_off = pp["C"], pp["g_off"], pp["kk_off"]
                gts = [{}, {}]
                ohs = {}

                def issue(t, g):
                    cg = int(C[t, g])
                    if cg == 0:
                        gts[g][t] = None
                        return
                    pool = gpool0 if g == 0 else gpool1
                    gt = pool.tile([128, cg, 128], f16, tag=f"gt{g}",
                                   name=f"gt{g}_{t}")
                    o0 = int(g_off[t, g])
                    nc.gpsimd.dma_gather(
                        gt[:], tables[g],
                        idxsb[g][:, o0 * 8:(o0 + cg) * 8],
                        num_idxs=cg * 128, num_idxs_reg=cg * 128,
                        elem_size=128, single_packet=False,
                        queue_num=(t + 2 * g) % NQ,
                    )
                    gts[g][t] = gt

                for step in range(NT + stag):
                    if step < NT:
                        t = step
                        nbk = int(C[t].sum())
                        ohsb = ohpool.tile([128, nbk * 128], f8, tag="ohsb",
                                           name=f"ohsb_{t}")
                        nc.scalar.dma_start(
                            ohsb[:],
                            ohd_t.ap()[:, int(kk_off[t]) * 128:
                                       (int(kk_off[t]) + nbk) * 128])
                        ohs[t] = ohsb
                        issue(t, 0)
                        if stag == 0:
                            issue(t, 1)
                    if stag and step >= stag - 1 and step - (stag - 1) < NT:
                        issue(step - (stag - 1), 1)
                    tc_ = step - stag if stag else step
                    if tc_ < 0 or tc_ >= NT:
                        continue
                    t = tc_
                    nch = int(C[t].sum())
                    ohsb = ohs.pop(t)
                    ps = psA.tile([128, 128], f32, tag="psA")
                    nc.tensor.matmul(ps[:],
                                     loc_res[:, t * 128:(t + 1) * 128],
                                     identsb[:],
                                     start=True, stop=(nch == 0),
                                     skip_group_check=True)
                    k = 0
                    for g in range(2):
                        gt = gts[g].pop(t)
                        for j2 in range(int(C[t, g])):
                            nc.tensor.matmul(
                                ps[:], gt[:, j2, :],
                                ohsb[:, k * 128:(k + 1) * 128],
                                start=False, stop=(k == nch - 1),
                                skip_group_check=True)
                            k += 1

                    aggT = wpool.tile([128, 128], f16, tag="aggT")
                    nc.scalar.copy(aggT[:], ps[:])
                    psh = psH.tile([128, 128], f32, tag="psH")
                    nc.tensor.matmul(psh[:], aggT[:],
                                     W1sb[:] if is_conv1 else Wcsb[:],
                                     start=True, stop=True,
                                     skip_group_check=True)
                    if is_conv1:
                        # h = relu(dis*psh + b1); hs = dis*h
                        hti = wpool.tile([128, 128], f32, tag="hti")
                        nc.vector.scalar_tensor_tensor(
                            hti[:], psh[:], dis32sb[:, t:t + 1], b1sb[:],
                            mult, add)
                        hct = hsres[:, t * 128:(t + 1) * 128]
                        nc.scalar.activation(
                            hct, hti[:], mybir.ActivationFunctionType.Relu,
                            scale=dis32sb[:, t:t + 1])
                        if t < TSPLIT:
                            nc.sync.dma_start(
                                hcsA[t * 128:(t + 1) * 128, :], hct)
                        else:
                            nc.sync.dma_start(
                                hcsB[(t - TSPLIT) * 128:(t - TSPLIT + 1) * 128, :],
                                hct)
                        if mid_cb is not None and t == AGA_AT:
                            mid_cb()
                    else:
                        ot = wpool.tile([128, 128], f32, tag="ot")
                        nc.vector.scalar_tensor_tensor(
                            ot[:], psh[:], dis32sb[:, t:t + 1], bcsb[:],
                            mult, add)
                        nc.sync.dma_start(out_ml.ap()[t * 128:(t + 1) * 128, :],
                                          ot[:])

            def issue_agA():
                nc.gpsimd.collective_compute(
                    "AllGather", mybir.AluOpType.bypass,
                    replica_groups=[list(range(NCORES))],
                    ins=[hcsA.opt()], outs=[hcfA.opt()],
                )

            conv_pass(p1, [x2Rd[0:LO, :], x2Rd[LO:NPAD, :]], idx1sb, oh1d,
                      xres, True, mid_cb=issue_agA)

            nc.gpsimd.collective_compute(
                "AllGather", mybir.AluOpType.bypass,
                replica_groups=[list(range(NCORES))],
                ins=[hcsB.opt()], outs=[hcfB.opt()],
            )

            conv_pass(p2, [hcfA[:], hcfB[:]], idx2sb, oh2d, hsres, False,
                      stag=STAG)

    nc.compile()
    return nc


def kernel(x, edge_index, W1, b1, W_mu, b_mu, W_logstd, b_logstd):
    global LAST_RESULTS
    from concourse.bass_utils import run_bass_kernel_spmd

    x = np.asarray(x, dtype=np.float32)
    W1 = np.asarray(W1, dtype=np.float32)
    b1 = np.asarray(b1, dtype=np.float32)
    W_mu = np.asarray(W_mu, dtype=np.float32)
    b_mu = np.asarray(b_mu, dtype=np.float32)
    W_logstd = np.asarray(W_logstd, dtype=np.float32)
    b_logstd = np.asarray(b_logstd, dtype=np.float32)

    key = np.asarray(edge_index).tobytes()[:64] + np.asarray(edge_index).tobytes()[-64:]
    cached = _CACHE.get("k")
    if cached is not None and cached[0] == key:
        _, dis, meta, nc = cached
    else:
        dis, meta = _preprocess(edge_index)
        nc = _build_nc(meta)
        _CACHE["k"] = (key, dis, meta, nc)

    x2R = np.zeros((NPAD, 128), np.float16)
    x2R[:N] = (x * dis[:, None]).astype(np.float16)
    W1h = W1.astype(np.float16)
    Wch = np.concatenate([W_mu, W_logstd], axis=1).astype(np.float16)
    b1r = np.tile(b1[None, :], (128, 1)).astype(np.float32)
    bcr = np.tile(np.concatenate([b_mu, b_logstd])[None, :], (128, 1)).astype(np.float32)
    disP = np.zeros(NPAD, np.float32)
    disP[:N] = dis
    ident = np.zeros((128, 128), np.uint8)
    ident[np.arange(128), np.arange(128)] = 0x38
    ident = ident.view(ml_dtypes.float8_e4m3)

    in_maps = []
    for c in range(NCORES):
        s1, oh1 = meta["p1"]["per_core"][c]
        s2, oh2 = meta["p2"]["per_core"][c]
        disSh = disP[c * SH:(c + 1) * SH].reshape(NT, 128).T  # [128, NT]
        in_maps.append({
            "x2Rd": x2R,
            "xlocd": np.ascontiguousarray(x2R[c * SH:(c + 1) * SH]),
            "W1d": W1h, "Wcd": Wch, "b1rd": b1r, "bcrd": bcr,
            "disT32d": np.ascontiguousarray(disSh.astype(np.float32)),
            "identd": ident,
            "idx1g0": s1[0], "idx1g1": s1[1],
            "idx2g0": s2[0], "idx2g1": s2[1],
            "oh1d": oh1, "oh2d": oh2,
        })

    res = run_bass_kernel_spmd(nc, in_maps, core_ids=list(range(NCORES)),
                               trace=TRACE)
    LAST_RESULTS = res
    full = np.concatenate([res.results[c]["out_ml"] for c in range(NCORES)],
                          axis=0)[:N]
    mu = np.ascontiguousarray(full[:, :OUT])
    logstd = np.ascontiguousarray(full[:, OUT:])
    return (mu, logstd)


# revision 42
# speedup vs baseline: 1.1594x; 1.1594x over previous
"""GCN encoder (3x GCNConv sharing one normalized adjacency) on 8 TRN2 NeuronCores.

v3 design:
  - Fold sym-norm into per-node scales: pre-scale rows by dis, post-scale
    aggregates by dis[dst].
  - Conv1 gathers directly from a replicated row-major (x*dis) table in HBM
    and aggregates raw input rows TRANSPOSED (psum[feat,dst] += chunk.T@OH);
    W1 is applied once per dst tile afterwards. No dense pre-GEMM, no first
    AllGather.
  - dma_gather descriptor generation runs on one Q7 core pair per SWDGE
    queue (~7.9ns/desc); gathers rotate over 4 queues so 4 pairs generate
    concurrently. 256B random HBM reads then become the wall (~0.35-0.5
    accesses/ns); deep buffering (TB=2 tile batches, 6 gather bufs) keeps
    the SDMA queues full, and per-block source-sorting improves locality.
  - One-hot scatter matrices precomputed on the host in fp8e4 and streamed
    from HBM (no DVE is_equal).
  - Self loops leave the gather streams; each dst tile adds its local rows
    via one identity matmul.
  - The republish AllGather is split in two chunks (tiles 0-24 / 25-48 of
    each shard) so chunk A overlaps the tail of conv1 and pass-2 gathers on
    table A overlap AllGather B. Pass 2 has its own group split (by chunk
    table), idx streams, and one-hots.
  - mu and logstd share one pass: Wc = [W_mu | W_logstd].
"""

import numpy as np
import ml_dtypes

N = 50000
E = 800000
IN = 128
HID = 128
OUT = 64
NCORES = 8
SH = 6272                 # nodes per core (padded)
NPAD = SH * NCORES        # 50176
NT = SH // 128            # 49 dst tiles per core
LO = 32768                # rows in pass-1 "lo" table (int16 limit)
TSPLIT = 25               # pass-2 chunk A = tiles [0,25), B = [25,49)
RA = TSPLIT * 128         # 3200 rows per shard in chunk A
RB = SH - RA              # 3072 rows per shard in chunk B
NROWA = NCORES * RA       # 25600 (< 32767: int16 ok)
NROWB = NCORES * RB       # 24576
TB = 1                    # dst tiles per gather batch
NQ = 4                    # SWDGE queues
AGA_AT = 33               # issue AllGather-A after consuming this tile (pass 1)
STAG = 8                  # pass-2: issue gB(t) after gA(t+STAG-1)

TRACE = False             # test.py sets this for profiling runs
LAST_RESULTS = None       # test.py reads exec_time_ns from here

_CACHE = {}


def _build_streams(es_tab, t, dl, g, ngrp):
    """Build per-core padded gather streams + fp8 one-hots for one pass.

    es_tab: per-message index into its group's table
    t: dst tile; dl: dst lane; g: group id (0..ngrp-1)
    All arrays are lists per core. Returns dict with C [NT,ngrp], offsets,
    per-core idx streams (per group) and OH fp8 arrays.
    """
    cnts = np.zeros((NCORES, NT, ngrp), np.int64)
    ordered = []
    for c in range(NCORES):
        order = np.lexsort((es_tab[c], g[c], t[c]))  # by tile, grp, src (locality)
        e, tt, dd, gg = es_tab[c][order], t[c][order], dl[c][order], g[c][order]
        key = tt * ngrp + gg
        bc = np.bincount(key, minlength=NT * ngrp)
        cnts[c] = bc.reshape(NT, ngrp)
        ordered.append((e, tt, dd, gg, key))

    C = (cnts.max(axis=0) + 127) // 128            # [NT, ngrp]
    K = C.sum(axis=0).astype(np.int64)             # chunks per group stream
    KT = int(C.sum())
    g_off = np.concatenate([np.zeros((1, ngrp), np.int64),
                            np.cumsum(C, axis=0)[:-1]], axis=0)  # [NT, ngrp]
    kk_off = np.concatenate([[0], np.cumsum(C.sum(axis=1))[:-1]])

    per_core = []
    for c in range(NCORES):
        e, tt, dd, gg, key = ordered[c]
        blk_start = np.concatenate([[0], np.cumsum(cnts[c].reshape(-1))[:-1]])
        rank = np.arange(len(e)) - blk_start[key]
        pos = g_off[tt, gg] * 128 + rank           # position in group stream
        streams = []
        for gi in range(ngrp):
            s = np.zeros(int(K[gi]) * 128, np.int16)
            m = gg == gi
            s[pos[m]] = e[m].astype(np.int16)
            streams.append(np.tile(s.reshape(-1, 16).T, (8, 1)))  # [128, K*8]
        # chunk index: tile-major, groups in order within tile
        cum_in_tile = np.concatenate(
            [np.zeros((NT, 1), np.int64), np.cumsum(C, axis=1)[:, :-1]], axis=1)
        kk = kk_off[tt] + cum_in_tile[tt, gg] + rank // 128
        oh = np.zeros((128, KT * 128), np.uint8)
        oh[rank % 128, kk * 128 + dd] = 0x38       # 1.0 in fp8 e4m3
        per_core.append((streams, oh.view(ml_dtypes.float8_e4m3)))

    return dict(C=C, K=K, KT=KT, g_off=g_off, kk_off=kk_off,
                per_core=per_core, ngrp=ngrp)


def _preprocess(edge_index):
    src = np.asarray(edge_index[0]).astype(np.int64)
    dst = np.asarray(edge_index[1]).astype(np.int64)
    loop = np.arange(N, dtype=np.int64)
    dst_all = np.concatenate([dst, loop])

    deg = np.bincount(dst_all, minlength=N).astype(np.float32)
    dis = (1.0 / np.sqrt(deg)).astype(np.float32)

    es_by_core, t_by_core, dl_by_core = [], [], []
    for c in range(NCORES):
        m = (dst // SH) == c
        es = src[m]
        ed = dst[m] - c * SH
        es_by_core.append(es)
        t_by_core.append(ed >> 7)
        dl_by_core.append(ed & 127)

    # pass 1: table = x2R [NPAD,128]; groups lo (idx<LO) / hi
    g1 = [(e >= LO).astype(np.int64) for e in es_by_core]
    e1 = [np.where(e >= LO, e - LO, e) for e in es_by_core]
    p1 = _build_streams(e1, t_by_core, dl_by_core, g1, 2)

    # pass 2: tables hcfA [NROWA,128] / hcfB [NROWB,128]
    # node (c2, r) -> table A pos c2*RA + r if r < RA else B pos c2*RB + (r-RA)
    g2, e2 = [], []
    for c in range(NCORES):
        e = es_by_core[c]
        c2, r = e // SH, e % SH
        inB = (r >= RA).astype(np.int64)
        g2.append(inB)
        e2.append(np.where(inB == 1, c2 * RB + (r - RA), c2 * RA + r))
    p2 = _build_streams(e2, t_by_core, dl_by_core, g2, 2)

    batches = []
    t0 = 0
    while t0 < NT:
        t1 = min(t0 + TB, NT)
        batches.append((t0, t1))
        t0 = t1
    return dis, dict(p1=p1, p2=p2, batches=batches)


def _build_nc(meta):
    import concourse.bass as bass
    import concourse.bacc as bacc
    import concourse.mybir as mybir
    import concourse.tile as tile
    from concourse import library_config

    batches = meta["batches"]
    p1, p2 = meta["p1"], meta["p2"]

    f16 = mybir.dt.float16
    f32 = mybir.dt.float32
    f8 = mybir.dt.float8e4
    i16 = mybir.dt.int16
    mult = mybir.AluOpType.mult
    add = mybir.AluOpType.add

    nc = bacc.Bacc("TRN2", target_bir_lowering=False, debug=False,
                   enable_asserts=True, num_devices=NCORES,
                   num_swdge_queues=NQ)

    x2Rd = nc.dram_tensor("x2Rd", [NPAD, 128], f16, kind="ExternalInput")
    xlocd = nc.dram_tensor("xlocd", [SH, 128], f16, kind="ExternalInput")
    W1d = nc.dram_tensor("W1d", [128, 128], f16, kind="ExternalInput")
    Wcd = nc.dram_tensor("Wcd", [128, 128], f16, kind="ExternalInput")
    b1rd = nc.dram_tensor("b1rd", [128, 128], f32, kind="ExternalInput")
    bcrd = nc.dram_tensor("bcrd", [128, 128], f32, kind="ExternalInput")
    disT32d = nc.dram_tensor("disT32d", [128, NT], f32, kind="ExternalInput")
    identd = nc.dram_tensor("identd", [128, 128], f8, kind="ExternalInput")
    idx1 = [nc.dram_tensor(f"idx1g{g}", [128, int(p1["K"][g]) * 8], i16,
                           kind="ExternalInput") for g in range(2)]
    idx2 = [nc.dram_tensor(f"idx2g{g}", [128, int(p2["K"][g]) * 8], i16,
                           kind="ExternalInput") for g in range(2)]
    oh1d = nc.dram_tensor("oh1d", [128, p1["KT"] * 128], f8, kind="ExternalInput")
    oh2d = nc.dram_tensor("oh2d", [128, p2["KT"] * 128], f8, kind="ExternalInput")
    out_ml = nc.dram_tensor("out_ml", [SH, 128], f32, kind="ExternalOutput")

    with tile.TileContext(nc) as tc:
        with (
            tc.tile_pool(name="consts", bufs=1) as cpool,
            tc.tile_pool(name="work", bufs=4) as wpool,
            tc.tile_pool(name="oh", bufs=6) as ohpool,
            tc.tile_pool(name="g0", bufs=10) as gpool0,
            tc.tile_pool(name="g1", bufs=10) as gpool1,
            tc.tile_pool(name="psA", bufs=4, space="PSUM") as psA,
            tc.tile_pool(name="psH", bufs=2, space="PSUM") as psH,
            tc.tile_pool(name="dram", bufs=1, space="DRAM") as dpool,
        ):
            nc.gpsimd.load_library(library_config.mlp)

            W1sb = cpool.tile([128, 128], f16, tag="W1sb")
            Wcsb = cpool.tile([128, 128], f16, tag="Wcsb")
            b1sb = cpool.tile([128, 128], f32, tag="b1sb")
            bcsb = cpool.tile([128, 128], f32, tag="bcsb")
            dis32sb = cpool.tile([128, NT], f32, tag="dis32sb")
            identsb = cpool.tile([128, 128], f8, tag="identsb")
            idx1sb = [cpool.tile([128, int(p1["K"][g]) * 8], i16,
                                 tag=f"idx1g{g}", name=f"idx1sb{g}")
                      for g in range(2)]
            idx2sb = [cpool.tile([128, int(p2["K"][g]) * 8], i16,
                                 tag=f"idx2g{g}", name=f"idx2sb{g}")
                      for g in range(2)]

            xres = cpool.tile([128, NT * 128], f16, tag="xres")
            hsres = cpool.tile([128, NT * 128], f16, tag="hsres")

            nc.sync.dma_start(W1sb[:], W1d.ap())
            nc.sync.dma_start(Wcsb[:], Wcd.ap())
            nc.sync.dma_start(b1sb[:], b1rd.ap())
            nc.sync.dma_start(bcsb[:], bcrd.ap())
            nc.sync.dma_start(dis32sb[:], disT32d.ap())
            nc.sync.dma_start(identsb[:], identd.ap())
            for g in range(2):
                nc.sync.dma_start(idx1sb[g][:], idx1[g].ap())
                nc.sync.dma_start(idx2sb[g][:], idx2[g].ap())
            for t in range(NT):
                nc.sync.dma_start(xres[:, t * 128:(t + 1) * 128],
                                  xlocd[t * 128:(t + 1) * 128, :])

            hcsA = dpool.tile([RA, 128], f16, tag="hcsA")
            hcsB = dpool.tile([RB, 128], f16, tag="hcsB")
            hcfA = dpool.tile([NROWA, 128], f16, tag="hcfA", addr_space="Shared")
            hcfB = dpool.tile([NROWB, 128], f16, tag="hcfB", addr_space="Shared")

            def conv_pass(pp, tables, idxsb, ohd_t, loc_res, is_conv1,
                          mid_cb=None, stag=0):
                C, g_off, kk_off = pp["C"], pp["g_off"], pp["kk_off"]
                gts = [{}, {}]
                ohs = {}

                def issue(t, g):
                    cg = int(C[t, g])
                    if cg == 0:
                        gts[g][t] = None
                        return
                    pool = gpool0 if g == 0 else gpool1
                    gt = pool.tile([128, cg, 128], f16, tag=f"gt{g}",
                                   name=f"gt{g}_{t}")
                    o0 = int(g_off[t, g])
                    nc.gpsimd.dma_gather(
                        gt[:], tables[g],
                        idxsb[g][:, o0 * 8:(o0 + cg) * 8],
                        num_idxs=cg * 128, num_idxs_reg=cg * 128,
                        elem_size=128, single_packet=False,
                        queue_num=(t + 2 * g) % NQ,
                    )
                    gts[g][t] = gt

                for step in range(NT + stag):
                    if step < NT:
                        t = step
                        nbk = int(C[t].sum())
                        ohsb = ohpool.tile([128, nbk * 128], f8, tag="ohsb",
                                           name=f"ohsb_{t}")
                        nc.scalar.dma_start(
                            ohsb[:],
                            ohd_t.ap()[:, int(kk_off[t]) * 128:
                                       (int(kk_off[t]) + nbk) * 128])
                        ohs[t] = ohsb
                        issue(t, 0)
                        if stag == 0:
                            issue(t, 1)
                    if stag and step >= stag - 1 and step - (stag - 1) < NT:
                        issue(step - (stag - 1), 1)
                    tc_ = step - stag if stag else step
                    if tc_ < 0 or tc_ >= NT:
                        continue
                    t = tc_
                    nch = int(C[t].sum())
                    ohsb = ohs.pop(t)
                    ps = psA.tile([128, 128], f32, tag="psA")
                    nc.tensor.matmul(ps[:],
                                     loc_res[:, t * 128:(t + 1) * 128],
                                     identsb[:],
                                     start=True, stop=(nch == 0),
                                     skip_group_check=True)
                    k = 0
                    for g in range(2):
                        gt = gts[g].pop(t)
                        for j2 in range(int(C[t, g])):
                            nc.tensor.matmul(
                                ps[:], gt[:, j2, :],
                                ohsb[:, k * 128:(k + 1) * 128],
                                start=False, stop=(k == nch - 1),
                                skip_group_check=True)
                            k += 1

                    aggT = wpool.tile([128, 128], f16, tag="aggT")
                    nc.scalar.copy(aggT[:], ps[:])
                    psh = psH.tile([128, 128], f32, tag="psH")
                    nc.tensor.matmul(psh[:], aggT[:],
                                     W1sb[:] if is_conv1 else Wcsb[:],
                                     start=True, stop=True,
                                     skip_group_check=True)
                    if is_conv1:
                        # h = relu(dis*psh + b1); hs = dis*h
                        hti = wpool.tile([128, 128], f32, tag="hti")
                        nc.vector.scalar_tensor_tensor(
                            hti[:], psh[:], dis32sb[:, t:t + 1], b1sb[:],
                            mult, add)
                        hct = hsres[:, t * 128:(t + 1) * 128]
                        nc.scalar.activation(
                            hct, hti[:], mybir.ActivationFunctionType.Relu,
                            scale=dis32sb[:, t:t + 1])
                        if t < TSPLIT:
                            nc.sync.dma_start(
                                hcsA[t * 128:(t + 1) * 128, :], hct)
                        else:
                            nc.sync.dma_start(
                                hcsB[(t - TSPLIT) * 128:(t - TSPLIT + 1) * 128, :],
                                hct)
                        if mid_cb is not None and t == AGA_AT:
                            mid_cb()
                    else:
                        ot = wpool.tile([128, 128], f32, tag="ot")
                        nc.vector.scalar_tensor_tensor(
                            ot[:], psh[:], dis32sb[:, t:t + 1], bcsb[:],
                            mult, add)
                        nc.sync.dma_start(out_ml.ap()[t * 128:(t + 1) * 128, :],
                                          ot[:])

            def issue_agA():
                nc.gpsimd.collective_compute(
                    "AllGather", mybir.AluOpType.bypass,
                    replica_groups=[list(range(NCORES))],
                    ins=[hcsA.opt()], outs=[hcfA.opt()],
                )

            conv_pass(p1, [x2Rd[0:LO, :], x2Rd[LO:NPAD, :]], idx1sb, oh1d,
                      xres, True, mid_cb=issue_agA)

            nc.gpsimd.collective_compute(
                "AllGather", mybir.AluOpType.bypass,
                replica_groups=[list(range(NCORES))],
                ins=[hcsB.opt()], outs=[hcfB.opt()],
            )

            conv_pass(p2, [hcfA[:], hcfB[:]], idx2sb, oh2d, hsres, False,
                      stag=STAG)

    nc.compile()
    return nc


def kernel(x, edge_index, W1, b1, W_mu, b_mu, W_logstd, b_logstd):
    global LAST_RESULTS
    from concourse.bass_utils import run_bass_kernel_spmd

    x = np.asarray(x, dtype=np.float32)
    W1 = np.asarray(W1, dtype=np.float32)
    b1 = np.asarray(b1, dtype=np.float32)
    W_mu = np.asarray(W_mu, dtype=np.float32)
    b_mu = np.asarray(b_mu, dtype=np.float32)
    W_logstd = np.asarray(W_logstd, dtype=np.float32)
    b_logstd = np.asarray(b_logstd, dtype=np.float32)

    key = np.asarray(edge_index).tobytes()[:64] + np.asarray(edge_index).tobytes()[-64:]
    cached = _CACHE.get("k")
    if cached is not None and cached[0] == key:
        _, dis, meta, nc = cached
    else:
        dis, meta = _preprocess(edge_index)
        nc = _build_nc(meta)
        _CACHE["k"] = (key, dis, meta, nc)

    x2R = np.zeros((NPAD, 128), np.float16)
    x2R[:N] = (x * dis[:, None]).astype(np.float16)
    W1h = W1.astype(np.float16)
    Wch = np.concatenate([W_mu, W_logstd], axis=1).astype(np.float16)
    b1r = np.tile(b1[None, :], (128, 1)).astype(np.float32)
    bcr = np.tile(np.concatenate([b_mu, b_logstd])[None, :], (128, 1)).astype(np.float32)
    disP = np.zeros(NPAD, np.float32)
    disP[:N] = dis
    ident = np.zeros((128, 128), np.uint8)
    ident[np.arange(128), np.arange(128)] = 0x38
    ident = ident.view(ml_dtypes.float8_e4m3)

    in_maps = []
    for c in range(NCORES):
        s1, oh1 = meta["p1"]["per_core"][c]
        s2, oh2 = meta["p2"]["per_core"][c]
        disSh = disP[c * SH:(c + 1) * SH].reshape(NT, 128).T  # [128, NT]
        in_maps.append({
            "x2Rd": x2R,
            "xlocd": np.ascontiguousarray(x2R[c * SH:(c + 1) * SH]),
            "W1d": W1h, "Wcd": Wch, "b1rd": b1r, "bcrd": bcr,
            "disT32d": np.ascontiguousarray(disSh.astype(np.float32)),
            "identd": ident,
            "idx1g0": s1[0], "idx1g1": s1[1],
            "idx2g0": s2[0], "idx2g1": s2[1],
            "oh1d": oh1, "oh2d": oh2,
        })

    res = run_bass_kernel_spmd(nc, in_maps, core_ids=list(range(NCORES)),
                               trace=TRACE)
    LAST_RESULTS = res
    full = np.concatenate([res.results[c]["out_ml"] for c in range(NCORES)],
                          axis=0)[:N]
    mu = np.ascontiguousarray(full[:, :OUT])
    logstd = np.ascontiguousarray(full[:, OUT:])
    return (mu, logstd)
